# revision 18
# baseline (speedup 1.0000x reference)
"""Trainium2 Bass kernel for DeformableAttentionV2 — v6 (tunable).

Baseline v2 structure with knobs set from HW probe results:
  GATHER_COLS  — points per indirect_dma_start (offset AP [128, k]).
  ACT_PTS      — kv corner-product points on ACT (rest on DVE).
  ACT_FIN      — final-combine products on ACT (activation Copy w/ scale).
  GP_FIN       — final-combine products on GPSIMD via tensor_tensor with a
                 stride-0 broadcast scalar (InstTensorTensor is in the
                 standard ucode library, unlike InstTensorScalar).
  GP_WCORN     — build bilinear corner weights on GPSIMD tensor_tensor.
1/Z is folded into the exs scale so the final full-width rescale is gone;
kv_pool/fin_pool are double-buffered to remove tile-to-tile serialization.
"""

import numpy as np
from contextlib import ExitStack

import concourse.bass as bass
import concourse.bacc as bacc_mod
import concourse.mybir as mybir
from concourse.bass import IndirectOffsetOnAxis
from concourse.tile import TileContext

# ---- hardcoded problem geometry ----
NQ = 20000
C = 256
NLEV = 2
NPT = 9
NLP = NLEV * NPT               # 18
H = W = 128
PAD = 4
HP = WP = H + 2 * PAD          # 136
LVL_ROWS = HP * WP             # 18496
VP_ROWS = NLEV * LVL_ROWS      # 36992
N_CORES = 8
NQ_CORE = 2560
NQ_REAL = 2500
P = 128
NOFF = NLP * 2                 # 36
MAGIC = float(np.float32(2.0 ** 23))
NT_FULL = NQ_CORE // P         # 20
NSC = 3                        # scalar consts: -16s, +16s, s
CONST_FREE = 2 * NOFF + NT_FULL * NOFF + NSC

F32 = mybir.dt.float32
F16 = mybir.dt.float16
I16 = mybir.dt.int16
I32 = mybir.dt.int32
AL = mybir.AluOpType
AX = mybir.AxisListType
AF = mybir.ActivationFunctionType

GATHER_COLS = 1      # points per indirect DMA (probe-verified value)
ACT_PTS = 12         # kv products for points [0, ACT_PTS) on ACT
ACT_FIN = 2          # final products [0, ACT_FIN) on ACT
GP_FIN = 0           # final products [ACT_FIN, ACT_FIN+GP_FIN) on GPSIMD
GP_WCORN = False     # corner-weight outer product on GPSIMD


def build_nc(n_tiles=NT_FULL, stop_after=None):
    nc = bacc_mod.Bacc()

    q_in = nc.declare_dram_parameter("q", [NQ_CORE, C], F32, isOutput=False)
    qt_in = nc.declare_dram_parameter("qt", [2 * P, NQ_CORE], F32, isOutput=False)
    consts_in = nc.declare_dram_parameter("consts", [P, CONST_FREE], F32,
                                          isOutput=False)
    vp_in = nc.declare_dram_parameter("vp", [VP_ROWS, 2 * C], I16, isOutput=False)
    out_ext = nc.declare_dram_parameter("out", [NQ_CORE, C], F32, isOutput=True)

    ctx = ExitStack()
    with ctx:
        tc = ctx.enter_context(TileContext(nc))

        const_pool = ctx.enter_context(tc.tile_pool(name="const", bufs=1))
        io_pool = ctx.enter_context(tc.tile_pool(name="io", bufs=2))
        g_pool = ctx.enter_context(tc.tile_pool(name="g", bufs=2))
        aprod_pool = ctx.enter_context(tc.tile_pool(name="aprod", bufs=2))
        dprod_pool = ctx.enter_context(tc.tile_pool(name="dprod", bufs=1))
        kv_pool = ctx.enter_context(tc.tile_pool(name="kv", bufs=2))
        sm_pool = ctx.enter_context(tc.tile_pool(name="sm", bufs=2))
        fin_pool = ctx.enter_context(tc.tile_pool(name="fin", bufs=2))
        ps_pool = ctx.enter_context(tc.tile_pool(name="ps", bufs=2, space="PSUM"))
        one_pool = ctx.enter_context(tc.tile_pool(name="one", bufs=1))

        consts_sb = const_pool.tile([P, CONST_FREE], F32, name="consts_sb")
        nc.sync.dma_start(out=consts_sb[:], in_=consts_in[:, :])
        w_sb = consts_sb[:, 0:2 * NOFF].rearrange("p (k j) -> p k j", k=2)
        refpx_full = consts_sb[:, 2 * NOFF:2 * NOFF + NT_FULL * NOFF].rearrange(
            "p (t j) -> p t j", j=NOFF)
        sc = 2 * NOFF + NT_FULL * NOFF
        neg16s = consts_sb[:, sc:sc + 1]
        pos16s = consts_sb[:, sc + 1:sc + 2]
        s_ap = consts_sb[:, sc + 2:sc + 3]

        scr_v = one_pool.tile([P, C], F32, name="scr_v")

        def emit_final(st):
            """Final combine: 18 fp16 products (ACT/GP/DVE split), fp16
            tree-add (2x) on DVE, fp32 cast store. 1/Z already in exs."""
            kv, exs, t = st
            fpr = fin_pool.tile([P, NLP, C], F16, name="fpr")
            for pt in range(NLP):
                if pt < ACT_FIN:
                    nc.scalar.activation(fpr[:, pt, :], kv[:, pt, :],
                                         AF.Copy, scale=exs[:, pt:pt + 1])
                elif pt < ACT_FIN + GP_FIN:
                    nc.gpsimd.tensor_tensor(
                        fpr[:, pt, :], kv[:, pt, :],
                        exs[:, pt:pt + 1].broadcast_to([P, C]), AL.mult)
                else:
                    nc.vector.tensor_scalar(fpr[:, pt, :], kv[:, pt, :],
                                            exs[:, pt:pt + 1], None, AL.mult)
            # pairwise tree (fp16 2x): 18 -> 9 -> 4 (+1 leftover) -> 2 -> 1
            t9 = fin_pool.tile([P, 9, C], F16, name="t9")
            nc.vector.tensor_tensor(
                t9[:], fpr[:, 0:18:2, :], fpr[:, 1:18:2, :], AL.add)
            t4 = fin_pool.tile([P, 4, C], F16, name="t4")
            nc.vector.tensor_tensor(
                t4[:], t9[:, 0:8:2, :], t9[:, 1:8:2, :], AL.add)
            t2 = fin_pool.tile([P, 2, C], F16, name="t2")
            nc.vector.tensor_tensor(
                t2[:], t4[:, 0:4:2, :], t4[:, 1:4:2, :], AL.add)
            t1 = fin_pool.tile([P, C], F16, name="t1")
            nc.vector.tensor_tensor(t1[:], t2[:, 0, :], t2[:, 1, :], AL.add)
            t1b = fin_pool.tile([P, C], F16, name="t1b")
            nc.vector.tensor_tensor(t1b[:], t1[:], t9[:, 8, :], AL.add)
            out_f = fin_pool.tile([P, C], F32, name="out_f")
            nc.vector.tensor_copy(out_f[:], t1b[:])
            nc.sync.dma_start(out=out_ext[bass.ts(t, P), :], in_=out_f[:])

        def emit_coords(t):
            """Tile t: input DMAs, offsets matmul, coords/floor, weights,
            int32 gather indices."""
            q_sb = io_pool.tile([P, C], F32, name="q_sb")
            nc.sync.dma_start(out=q_sb[:], in_=q_in[bass.ts(t, P), :])
            qt_sb = io_pool.tile([P, 2, P], F32, name="qt_sb")
            nc.sync.dma_start(
                out=qt_sb[:],
                in_=qt_in[:, :].rearrange("(k c) q -> c k q", k=2)[:, :, bass.ts(t, P)],
            )

            off_ps = ps_pool.tile([P, NOFF], F32, space="PSUM", name="off_ps")
            for k in range(2):
                nc.tensor.matmul(
                    out=off_ps[:], lhsT=qt_sb[:, k, :], rhs=w_sb[:, k, :],
                    start=(k == 0), stop=(k == 1))

            coord = sm_pool.tile([P, NOFF], F32, name="coord")
            nc.vector.tensor_tensor(coord[:], off_ps[:], refpx_full[:, t, :], AL.add)

            # floor via 2^23 magic
            rnd = sm_pool.tile([P, NOFF], F32, name="rnd")
            nc.vector.tensor_scalar(rnd[:], coord[:], MAGIC, MAGIC,
                                    AL.add, AL.subtract)
            gt = sm_pool.tile([P, NOFF], F32, name="gt")
            nc.vector.tensor_tensor(gt[:], rnd[:], coord[:], AL.is_gt)
            fl = sm_pool.tile([P, NOFF], F32, name="fl")
            nc.vector.tensor_tensor(fl[:], rnd[:], gt[:], AL.subtract)
            nc.vector.tensor_scalar(fl[:], fl[:], 0.0, 134.0, AL.max, AL.min)
            frac = sm_pool.tile([P, NOFF], F32, name="frac")
            nc.vector.tensor_tensor(frac[:], coord[:], fl[:], AL.subtract)

            wx01 = sm_pool.tile([P, NLP, 2], F32, name="wx01")
            wy01 = sm_pool.tile([P, NLP, 2], F32, name="wy01")
            for arr, xy in ((wx01, 0), (wy01, 1)):
                fr = frac[:, xy:NOFF:2]
                nc.vector.tensor_scalar(arr[:, :, 0], fr, -1.0, 1.0,
                                        AL.mult, AL.add)
                nc.vector.tensor_copy(arr[:, :, 1], fr)
            wcorn = sm_pool.tile([P, NLP, 4], F32, name="wcorn")
            weng = nc.gpsimd if GP_WCORN else nc.vector
            weng.tensor_tensor(
                wcorn[:].rearrange("p k (y x) -> p k y x", y=2, x=2),
                wy01[:, :, :, None].broadcast_to([P, NLP, 2, 2]),
                wx01[:, :, None, :].broadcast_to([P, NLP, 2, 2]),
                AL.mult,
            )

            basef = sm_pool.tile([P, NLP], F32, name="basef")
            nc.vector.tensor_scalar(basef[:], fl[:, 1:NOFF:2], 136.0, None, AL.mult)
            nc.vector.tensor_tensor(basef[:], basef[:], fl[:, 0:NOFF:2], AL.add)
            nc.vector.tensor_scalar(basef[:, NPT:], basef[:, NPT:],
                                    float(LVL_ROWS), None, AL.add)
            idxi = sm_pool.tile([P, NLP], I32, name="idxi")
            nc.vector.tensor_copy(idxi[:], basef[:])
            return (idxi, wcorn, q_sb, t)

        def emit_gather(co):
            """Indirect gathers, GATHER_COLS points per instruction."""
            idxi, wcorn, q_sb, t = co
            g_sb = g_pool.tile([P, NLP, 1024], I16, name="g_sb")
            for p0 in range(0, NLP, GATHER_COLS):
                p1 = min(p0 + GATHER_COLS, NLP)
                if p1 - p0 == 1:
                    nc.gpsimd.indirect_dma_start(
                        out=g_sb[:, p0, :], out_offset=None, in_=vp_in[:, :],
                        in_offset=IndirectOffsetOnAxis(
                            ap=idxi[:, p0:p0 + 1], axis=0))
                else:
                    nc.gpsimd.indirect_dma_start(
                        out=g_sb[:, p0:p1, :], out_offset=None, in_=vp_in[:, :],
                        in_offset=IndirectOffsetOnAxis(
                            ap=idxi[:, p0:p1], axis=0))
            return (g_sb, wcorn, q_sb, t)

        def g_slice(g_sb, pt, dy, dx):
            return g_sb[:, pt, 512 * dx + 256 * dy:512 * dx + 256 * dy + 256]

        def emit_act_products(head):
            """ACT's share of the kv corner products for a tile, emitted one
            tile ahead (after the previous tile's exp) so ACT never stalls."""
            g_sb, wcorn, _, _ = head
            aprod = aprod_pool.tile([P, ACT_PTS, 2, 2, C], I16, name="aprod")
            for pt in range(ACT_PTS):
                for dy in range(2):
                    for dx in range(2):
                        nc.scalar.activation(
                            aprod[:, pt, dy, dx, :], g_slice(g_sb, pt, dy, dx),
                            AF.Copy, scale=wcorn[:, pt, 2 * dy + dx:
                                                 2 * dy + dx + 1])
            return aprod

        def emit_compute(head, aprod):
            g_sb, wcorn, q_sb, t = head

            # DVE's share of the corner products (points ACT_PTS..17)
            nd = NLP - ACT_PTS
            dprod = dprod_pool.tile([P, nd, 2, 2, C], I16, name="dprod")
            for i, pt in enumerate(range(ACT_PTS, NLP)):
                for dy in range(2):
                    for dx in range(2):
                        nc.vector.tensor_scalar(
                            dprod[:, i, dy, dx, :], g_slice(g_sb, pt, dy, dx),
                            wcorn[:, pt, 2 * dy + dx:2 * dy + dx + 1],
                            None, AL.mult)

            # kv accumulation: 3 chained int16 TT adds per buffer (exact)
            kv = kv_pool.tile([P, NLP, C], I16, name="kv")
            for buf, p0, np_ in ((aprod, 0, ACT_PTS), (dprod, ACT_PTS, nd)):
                kvs = kv[:, p0:p0 + np_, :]
                nc.vector.tensor_tensor(kvs, buf[:, :, 0, 0, :],
                                        buf[:, :, 0, 1, :], AL.add)
                nc.vector.tensor_tensor(kvs, kvs, buf[:, :, 1, 0, :], AL.add)
                nc.vector.tensor_tensor(kvs, kvs, buf[:, :, 1, 1, :], AL.add)

            # logits + softmax
            lg = sm_pool.tile([P, NLP], F32, name="lg")
            for pt in range(NLP):
                nc.vector.scalar_tensor_tensor(
                    out=scr_v[:], in0=kv[:, pt, :], scalar=1.0, in1=q_sb[:],
                    op0=AL.mult, op1=AL.mult, accum_out=lg[:, pt:pt + 1])
            mx = sm_pool.tile([P, 1], F32, name="mx")
            nc.vector.tensor_reduce(mx[:], lg[:], AX.X, AL.max)
            nbias = sm_pool.tile([P, 1], F32, name="nbias")
            nc.vector.tensor_scalar(nbias[:], mx[:], neg16s, None, AL.mult)
            ex = sm_pool.tile([P, NLP], F32, name="ex")
            nc.scalar.activation(ex[:], lg[:], AF.Exp, bias=nbias[:],
                                 scale=pos16s)
            rs = sm_pool.tile([P, 1], F32, name="rs")
            nc.vector.tensor_reduce(rs[:], ex[:], AX.X, AL.add)
            rinv = sm_pool.tile([P, 1], F32, name="rinv")
            nc.vector.reciprocal(rinv[:], rs[:])
            # sr = s * (1/Z): per-partition scale folded into exs
            sr = sm_pool.tile([P, 1], F32, name="sr")
            nc.vector.tensor_scalar(sr[:], rinv[:], s_ap, None, AL.mult)
            exs = sm_pool.tile([P, NLP], F32, name="exs")
            nc.scalar.activation(exs[:], ex[:], AF.Copy, scale=sr[:, 0:1])
            return (kv, exs, t)

        co = emit_coords(0)
        head = emit_gather(co)
        aprod = emit_act_products(head)
        for t in range(n_tiles):
            nxt_co = emit_coords(t + 1) if t + 1 < n_tiles else None
            st = emit_compute(head, aprod)
            if nxt_co is not None:
                nxt = emit_gather(nxt_co)
            if st is not None:
                emit_final(st)
            if nxt_co is not None:
                nxt_aprod = emit_act_products(nxt)
                head, aprod = nxt, nxt_aprod
    return nc


# ---------------- host side ----------------

def host_prep(query, value, reference_points, W_off, b_off):
    rp_all = np.asarray(reference_points[0], dtype=np.float32)
    # sort queries by level-0 sample address so each gather's 128
    # descriptors hit a narrow, mostly-ascending band of the table
    # (HBM row locality); output is scattered back to original order.
    key = (np.rint(128.0 * rp_all[:, 0, 1]) * 136.0 + 128.0 * rp_all[:, 0, 0])
    order = np.argsort(key, kind="stable").astype(np.int64)
    q = np.ascontiguousarray(query[0][order], dtype=np.float32)
    rp = np.ascontiguousarray(rp_all[order])

    v = np.asarray(value, np.float32)
    s = float(np.abs(v).max() / 32766.0)
    vpad = np.zeros((NLEV, HP, WP, C), np.float32)
    vpad[:, PAD:PAD + H, PAD:PAD + W, :] = v.reshape(NLEV, H, W, C)
    vq = np.clip(np.rint(vpad * np.float32(1.0 / s)), -32767, 32767)
    vq = vq.astype(np.int16)
    vp = np.zeros((NLEV, HP, WP, 2, C), np.int16)
    vp[:, :, :, 0, :] = vq
    vp[:, :-1, :, 1, :] = vq[:, 1:]
    vp = np.ascontiguousarray(vp.reshape(VP_ROWS, 2 * C))

    b = np.asarray(b_off, np.float32).reshape(NLEV, NPT, 2)
    refpx = (np.float32(128.0) * rp[:, :, None, :] + b[None] + np.float32(3.5))
    refpx = np.ascontiguousarray(refpx.reshape(NQ, NOFF), np.float32)

    woff = np.ascontiguousarray(W_off, np.float32)
    return q, vp, refpx, woff, s, order


def pack_consts(woff, refpx_core, s):
    consts = np.empty((P, CONST_FREE), np.float32)
    consts[:, :2 * NOFF] = woff.reshape(2, P, NOFF).transpose(1, 0, 2).reshape(P, -1)
    consts[:, 2 * NOFF:2 * NOFF + NT_FULL * NOFF] = (
        refpx_core.reshape(NT_FULL, P, NOFF).transpose(1, 0, 2).reshape(P, -1))
    sc = 2 * NOFF + NT_FULL * NOFF
    consts[:, sc] = -16.0 * s
    consts[:, sc + 1] = 16.0 * s
    consts[:, sc + 2] = s
    return consts


def shard(q, refpx, woff, s):
    qs, cs, qts = [], [], []
    for c in range(N_CORES):
        sl = slice(c * NQ_REAL, (c + 1) * NQ_REAL)
        qp = np.zeros((NQ_CORE, C), np.float32)
        qp[:NQ_REAL] = q[sl]
        rp = np.full((NQ_CORE, NOFF), 67.5, np.float32)
        rp[:NQ_REAL] = refpx[sl]
        qs.append(qp)
        cs.append(pack_consts(woff, rp, s))
        qts.append(np.ascontiguousarray(qp.T))
    return qs, cs, qts


_NC_CACHE = {}


def kernel(query, key, value, reference_points, spatial_shapes, W_off, b_off):
    from concourse.bass_utils import run_bass_kernel_spmd

    q, vp, refpx, woff, s, order = host_prep(query, value, reference_points,
                                             W_off, b_off)
    qs, cs, qts = shard(q, refpx, woff, s)

    if "nc" not in _NC_CACHE:
        nc = build_nc()
        nc.finalize()
        _NC_CACHE["nc"] = nc
    nc = _NC_CACHE["nc"]

    in_maps = [
        {"q": qs[c], "qt": qts[c], "consts": cs[c], "vp": vp}
        for c in range(N_CORES)
    ]
    res = run_bass_kernel_spmd(nc, in_maps, list(range(N_CORES)))
    srt = np.concatenate([res.results[c]["out"][:NQ_REAL] for c in range(N_CORES)], 0)
    out = np.empty_like(srt)
    out[order] = srt
    return out[None].astype(np.float32)


# revision 20
# speedup vs baseline: 1.0101x; 1.0101x over previous
"""Trainium2 Bass kernel for DeformableAttentionV2 — v6 (tunable).

Baseline v2 structure with knobs set from HW probe results:
  GATHER_COLS  — points per indirect_dma_start (offset AP [128, k]).
  ACT_PTS      — kv corner-product points on ACT (rest on DVE).
  ACT_FIN      — final-combine products on ACT (activation Copy w/ scale).
  GP_FIN       — final-combine products on GPSIMD via tensor_tensor with a
                 stride-0 broadcast scalar (InstTensorTensor is in the
                 standard ucode library, unlike InstTensorScalar).
  GP_WCORN     — build bilinear corner weights on GPSIMD tensor_tensor.
1/Z is folded into the exs scale so the final full-width rescale is gone;
kv_pool/fin_pool are double-buffered to remove tile-to-tile serialization.
"""

import numpy as np
from contextlib import ExitStack

import concourse.bass as bass
import concourse.bacc as bacc_mod
import concourse.mybir as mybir
from concourse.bass import IndirectOffsetOnAxis
from concourse.tile import TileContext

# ---- hardcoded problem geometry ----
NQ = 20000
C = 256
NLEV = 2
NPT = 9
NLP = NLEV * NPT               # 18
H = W = 128
PAD = 4
HP = WP = H + 2 * PAD          # 136
LVL_ROWS = HP * WP             # 18496
VP_ROWS = NLEV * LVL_ROWS      # 36992
N_CORES = 8
NQ_CORE = 2560
NQ_REAL = 2500
P = 128
NOFF = NLP * 2                 # 36
MAGIC = float(np.float32(2.0 ** 23))
NT_FULL = NQ_CORE // P         # 20
NSC = 3                        # scalar consts: -16s, +16s, s
CONST_FREE = 2 * NOFF + NT_FULL * NOFF + NSC

F32 = mybir.dt.float32
F16 = mybir.dt.float16
I16 = mybir.dt.int16
I32 = mybir.dt.int32
AL = mybir.AluOpType
AX = mybir.AxisListType
AF = mybir.ActivationFunctionType

GATHER_COLS = 1      # points per indirect DMA (probe-verified value)
ACT_PTS = 12         # kv products for points [0, ACT_PTS) on ACT
ACT_FIN = 0          # final products [0, ACT_FIN) on ACT
GP_FIN = 0           # final products [ACT_FIN, ACT_FIN+GP_FIN) on GPSIMD
GP_WCORN = False     # corner-weight outer product on GPSIMD


def build_nc(n_tiles=NT_FULL, stop_after=None):
    nc = bacc_mod.Bacc()

    q_in = nc.declare_dram_parameter("q", [NQ_CORE, C], F32, isOutput=False)
    qt_in = nc.declare_dram_parameter("qt", [2 * P, NQ_CORE], F32, isOutput=False)
    consts_in = nc.declare_dram_parameter("consts", [P, CONST_FREE], F32,
                                          isOutput=False)
    vp_in = nc.declare_dram_parameter("vp", [VP_ROWS, 2 * C], I16, isOutput=False)
    out_ext = nc.declare_dram_parameter("out", [NQ_CORE, C], F32, isOutput=True)

    ctx = ExitStack()
    with ctx:
        tc = ctx.enter_context(TileContext(nc))

        const_pool = ctx.enter_context(tc.tile_pool(name="const", bufs=1))
        io_pool = ctx.enter_context(tc.tile_pool(name="io", bufs=2))
        g_pool = ctx.enter_context(tc.tile_pool(name="g", bufs=2))
        aprod_pool = ctx.enter_context(tc.tile_pool(name="aprod", bufs=2))
        dprod_pool = ctx.enter_context(tc.tile_pool(name="dprod", bufs=1))
        kv_pool = ctx.enter_context(tc.tile_pool(name="kv", bufs=2))
        sm_pool = ctx.enter_context(tc.tile_pool(name="sm", bufs=2))
        fin_pool = ctx.enter_context(tc.tile_pool(name="fin", bufs=2))
        ps_pool = ctx.enter_context(tc.tile_pool(name="ps", bufs=2, space="PSUM"))
        one_pool = ctx.enter_context(tc.tile_pool(name="one", bufs=1))

        consts_sb = const_pool.tile([P, CONST_FREE], F32, name="consts_sb")
        nc.sync.dma_start(out=consts_sb[:], in_=consts_in[:, :])
        w_sb = consts_sb[:, 0:2 * NOFF].rearrange("p (k j) -> p k j", k=2)
        refpx_full = consts_sb[:, 2 * NOFF:2 * NOFF + NT_FULL * NOFF].rearrange(
            "p (t j) -> p t j", j=NOFF)
        sc = 2 * NOFF + NT_FULL * NOFF
        neg16s = consts_sb[:, sc:sc + 1]
        pos16s = consts_sb[:, sc + 1:sc + 2]
        s_ap = consts_sb[:, sc + 2:sc + 3]

        scr_v = one_pool.tile([P, C], F32, name="scr_v")

        def emit_final(st):
            """Final combine: 18 fp16 products (ACT/GP/DVE split), fp16
            tree-add (2x) on DVE, fp32 cast store. 1/Z already in exs."""
            kv, exs, t = st
            fpr = fin_pool.tile([P, NLP, C], F16, name="fpr")
            for pt in range(NLP):
                if pt < ACT_FIN:
                    nc.scalar.activation(fpr[:, pt, :], kv[:, pt, :],
                                         AF.Copy, scale=exs[:, pt:pt + 1])
                elif pt < ACT_FIN + GP_FIN:
                    nc.gpsimd.tensor_tensor(
                        fpr[:, pt, :], kv[:, pt, :],
                        exs[:, pt:pt + 1].broadcast_to([P, C]), AL.mult)
                else:
                    nc.vector.tensor_scalar(fpr[:, pt, :], kv[:, pt, :],
                                            exs[:, pt:pt + 1], None, AL.mult)
            # pairwise tree (fp16 2x): 18 -> 9 -> 4 (+1 leftover) -> 2 -> 1
            t9 = fin_pool.tile([P, 9, C], F16, name="t9")
            nc.vector.tensor_tensor(
                t9[:], fpr[:, 0:18:2, :], fpr[:, 1:18:2, :], AL.add)
            t4 = fin_pool.tile([P, 4, C], F16, name="t4")
            nc.vector.tensor_tensor(
                t4[:], t9[:, 0:8:2, :], t9[:, 1:8:2, :], AL.add)
            t2 = fin_pool.tile([P, 2, C], F16, name="t2")
            nc.vector.tensor_tensor(
                t2[:], t4[:, 0:4:2, :], t4[:, 1:4:2, :], AL.add)
            t1 = fin_pool.tile([P, C], F16, name="t1")
            nc.vector.tensor_tensor(t1[:], t2[:, 0, :], t2[:, 1, :], AL.add)
            t1b = fin_pool.tile([P, C], F16, name="t1b")
            nc.vector.tensor_tensor(t1b[:], t1[:], t9[:, 8, :], AL.add)
            out_f = fin_pool.tile([P, C], F32, name="out_f")
            nc.vector.tensor_copy(out_f[:], t1b[:])
            nc.sync.dma_start(out=out_ext[bass.ts(t, P), :], in_=out_f[:])

        def emit_coords(t):
            """Tile t: input DMAs, offsets matmul, coords/floor, weights,
            int32 gather indices."""
            q_sb = io_pool.tile([P, C], F32, name="q_sb")
            nc.sync.dma_start(out=q_sb[:], in_=q_in[bass.ts(t, P), :])
            qt_sb = io_pool.tile([P, 2, P], F32, name="qt_sb")
            nc.sync.dma_start(
                out=qt_sb[:],
                in_=qt_in[:, :].rearrange("(k c) q -> c k q", k=2)[:, :, bass.ts(t, P)],
            )

            off_ps = ps_pool.tile([P, NOFF], F32, space="PSUM", name="off_ps")
            for k in range(2):
                nc.tensor.matmul(
                    out=off_ps[:], lhsT=qt_sb[:, k, :], rhs=w_sb[:, k, :],
                    start=(k == 0), stop=(k == 1))

            coord = sm_pool.tile([P, NOFF], F32, name="coord")
            nc.vector.tensor_tensor(coord[:], off_ps[:], refpx_full[:, t, :], AL.add)

            # floor via 2^23 magic
            rnd = sm_pool.tile([P, NOFF], F32, name="rnd")
            nc.vector.tensor_scalar(rnd[:], coord[:], MAGIC, MAGIC,
                                    AL.add, AL.subtract)
            gt = sm_pool.tile([P, NOFF], F32, name="gt")
            nc.vector.tensor_tensor(gt[:], rnd[:], coord[:], AL.is_gt)
            fl = sm_pool.tile([P, NOFF], F32, name="fl")
            nc.vector.tensor_tensor(fl[:], rnd[:], gt[:], AL.subtract)
            nc.vector.tensor_scalar(fl[:], fl[:], 0.0, 134.0, AL.max, AL.min)
            frac = sm_pool.tile([P, NOFF], F32, name="frac")
            nc.vector.tensor_tensor(frac[:], coord[:], fl[:], AL.subtract)

            wx01 = sm_pool.tile([P, NLP, 2], F32, name="wx01")
            wy01 = sm_pool.tile([P, NLP, 2], F32, name="wy01")
            for arr, xy in ((wx01, 0), (wy01, 1)):
                fr = frac[:, xy:NOFF:2]
                nc.vector.tensor_scalar(arr[:, :, 0], fr, -1.0, 1.0,
                                        AL.mult, AL.add)
                nc.vector.tensor_copy(arr[:, :, 1], fr)
            wcorn = sm_pool.tile([P, NLP, 4], F32, name="wcorn")
            weng = nc.gpsimd if GP_WCORN else nc.vector
            weng.tensor_tensor(
                wcorn[:].rearrange("p k (y x) -> p k y x", y=2, x=2),
                wy01[:, :, :, None].broadcast_to([P, NLP, 2, 2]),
                wx01[:, :, None, :].broadcast_to([P, NLP, 2, 2]),
                AL.mult,
            )

            basef = sm_pool.tile([P, NLP], F32, name="basef")
            nc.vector.tensor_scalar(basef[:], fl[:, 1:NOFF:2], 136.0, None, AL.mult)
            nc.vector.tensor_tensor(basef[:], basef[:], fl[:, 0:NOFF:2], AL.add)
            nc.vector.tensor_scalar(basef[:, NPT:], basef[:, NPT:],
                                    float(LVL_ROWS), None, AL.add)
            idxi = sm_pool.tile([P, NLP], I32, name="idxi")
            nc.vector.tensor_copy(idxi[:], basef[:])
            return (idxi, wcorn, q_sb, t)

        def emit_gather(co):
            """Indirect gathers, GATHER_COLS points per instruction."""
            idxi, wcorn, q_sb, t = co
            g_sb = g_pool.tile([P, NLP, 1024], I16, name="g_sb")
            for p0 in range(0, NLP, GATHER_COLS):
                p1 = min(p0 + GATHER_COLS, NLP)
                if p1 - p0 == 1:
                    nc.gpsimd.indirect_dma_start(
                        out=g_sb[:, p0, :], out_offset=None, in_=vp_in[:, :],
                        in_offset=IndirectOffsetOnAxis(
                            ap=idxi[:, p0:p0 + 1], axis=0))
                else:
                    nc.gpsimd.indirect_dma_start(
                        out=g_sb[:, p0:p1, :], out_offset=None, in_=vp_in[:, :],
                        in_offset=IndirectOffsetOnAxis(
                            ap=idxi[:, p0:p1], axis=0))
            return (g_sb, wcorn, q_sb, t)

        def g_slice(g_sb, pt, dy, dx):
            return g_sb[:, pt, 512 * dx + 256 * dy:512 * dx + 256 * dy + 256]

        def emit_act_products(head):
            """ACT's share of the kv corner products for a tile, emitted one
            tile ahead (after the previous tile's exp) so ACT never stalls."""
            g_sb, wcorn, _, _ = head
            aprod = aprod_pool.tile([P, ACT_PTS, 2, 2, C], I16, name="aprod")
            for pt in range(ACT_PTS):
                for dy in range(2):
                    for dx in range(2):
                        nc.scalar.activation(
                            aprod[:, pt, dy, dx, :], g_slice(g_sb, pt, dy, dx),
                            AF.Copy, scale=wcorn[:, pt, 2 * dy + dx:
                                                 2 * dy + dx + 1])
            return aprod

        def emit_kv_logits(head, aprod):
            """DVE: its corner products, exact kv adds, logit stt accums."""
            g_sb, wcorn, q_sb, t = head

            # DVE's share of the corner products (points ACT_PTS..17)
            nd = NLP - ACT_PTS
            dprod = dprod_pool.tile([P, nd, 2, 2, C], I16, name="dprod")
            for i, pt in enumerate(range(ACT_PTS, NLP)):
                for dy in range(2):
                    for dx in range(2):
                        nc.vector.tensor_scalar(
                            dprod[:, i, dy, dx, :], g_slice(g_sb, pt, dy, dx),
                            wcorn[:, pt, 2 * dy + dx:2 * dy + dx + 1],
                            None, AL.mult)

            # kv accumulation: 3 chained int16 TT adds per buffer (exact)
            kv = kv_pool.tile([P, NLP, C], I16, name="kv")
            for buf, p0, np_ in ((aprod, 0, ACT_PTS), (dprod, ACT_PTS, nd)):
                kvs = kv[:, p0:p0 + np_, :]
                nc.vector.tensor_tensor(kvs, buf[:, :, 0, 0, :],
                                        buf[:, :, 0, 1, :], AL.add)
                nc.vector.tensor_tensor(kvs, kvs, buf[:, :, 1, 0, :], AL.add)
                nc.vector.tensor_tensor(kvs, kvs, buf[:, :, 1, 1, :], AL.add)

            lg = sm_pool.tile([P, NLP], F32, name="lg")
            for pt in range(NLP):
                nc.vector.scalar_tensor_tensor(
                    out=scr_v[:], in0=kv[:, pt, :], scalar=1.0, in1=q_sb[:],
                    op0=AL.mult, op1=AL.mult, accum_out=lg[:, pt:pt + 1])
            return (kv, lg, t)

        def emit_softmax(kvlg):
            """Softmax; exp/exs run on ACT *behind* the next tile's products
            in queue order so ACT never idles waiting on DVE's logits."""
            kv, lg, t = kvlg
            mx = sm_pool.tile([P, 1], F32, name="mx")
            nc.vector.tensor_reduce(mx[:], lg[:], AX.X, AL.max)
            nbias = sm_pool.tile([P, 1], F32, name="nbias")
            nc.vector.tensor_scalar(nbias[:], mx[:], neg16s, None, AL.mult)
            ex = sm_pool.tile([P, NLP], F32, name="ex")
            nc.scalar.activation(ex[:], lg[:], AF.Exp, bias=nbias[:],
                                 scale=pos16s)
            rs = sm_pool.tile([P, 1], F32, name="rs")
            nc.vector.tensor_reduce(rs[:], ex[:], AX.X, AL.add)
            rinv = sm_pool.tile([P, 1], F32, name="rinv")
            nc.vector.reciprocal(rinv[:], rs[:])
            # sr = s * (1/Z): per-partition scale folded into exs
            sr = sm_pool.tile([P, 1], F32, name="sr")
            nc.vector.tensor_scalar(sr[:], rinv[:], s_ap, None, AL.mult)
            exs = sm_pool.tile([P, NLP], F32, name="exs")
            nc.scalar.activation(exs[:], ex[:], AF.Copy, scale=sr[:, 0:1])
            return (kv, exs, t)

        co = emit_coords(0)
        head = emit_gather(co)
        aprod = emit_act_products(head)
        for t in range(n_tiles):
            nxt_co = emit_coords(t + 1) if t + 1 < n_tiles else None
            kvlg = emit_kv_logits(head, aprod)
            # ACT products for t+1 queue BEFORE exp(t): ACT streams products
            # while DVE runs adds/logits — breaks the products->logits->exp
            # loop-carried chain that bounded v2 at ~35.4us/tile.
            if nxt_co is not None:
                nxt = emit_gather(nxt_co)
                nxt_aprod = emit_act_products(nxt)
            st = emit_softmax(kvlg)
            emit_final(st)
            if nxt_co is not None:
                head, aprod = nxt, nxt_aprod
    return nc


# ---------------- host side ----------------

def host_prep(query, value, reference_points, W_off, b_off):
    rp_all = np.asarray(reference_points[0], dtype=np.float32)
    # sort queries by level-0 sample address so each gather's 128
    # descriptors hit a narrow, mostly-ascending band of the table
    # (HBM row locality); output is scattered back to original order.
    key = (np.rint(128.0 * rp_all[:, 0, 1]) * 136.0 + 128.0 * rp_all[:, 0, 0])
    order = np.argsort(key, kind="stable").astype(np.int64)
    q = np.ascontiguousarray(query[0][order], dtype=np.float32)
    rp = np.ascontiguousarray(rp_all[order])

    v = np.asarray(value, np.float32)
    s = float(np.abs(v).max() / 32766.0)
    vpad = np.zeros((NLEV, HP, WP, C), np.float32)
    vpad[:, PAD:PAD + H, PAD:PAD + W, :] = v.reshape(NLEV, H, W, C)
    vq = np.clip(np.rint(vpad * np.float32(1.0 / s)), -32767, 32767)
    vq = vq.astype(np.int16)
    vp = np.zeros((NLEV, HP, WP, 2, C), np.int16)
    vp[:, :, :, 0, :] = vq
    vp[:, :-1, :, 1, :] = vq[:, 1:]
    vp = np.ascontiguousarray(vp.reshape(VP_ROWS, 2 * C))

    b = np.asarray(b_off, np.float32).reshape(NLEV, NPT, 2)
    refpx = (np.float32(128.0) * rp[:, :, None, :] + b[None] + np.float32(3.5))
    refpx = np.ascontiguousarray(refpx.reshape(NQ, NOFF), np.float32)

    woff = np.ascontiguousarray(W_off, np.float32)
    return q, vp, refpx, woff, s, order


def pack_consts(woff, refpx_core, s):
    consts = np.empty((P, CONST_FREE), np.float32)
    consts[:, :2 * NOFF] = woff.reshape(2, P, NOFF).transpose(1, 0, 2).reshape(P, -1)
    consts[:, 2 * NOFF:2 * NOFF + NT_FULL * NOFF] = (
        refpx_core.reshape(NT_FULL, P, NOFF).transpose(1, 0, 2).reshape(P, -1))
    sc = 2 * NOFF + NT_FULL * NOFF
    consts[:, sc] = -16.0 * s
    consts[:, sc + 1] = 16.0 * s
    consts[:, sc + 2] = s
    return consts


def shard(q, refpx, woff, s):
    qs, cs, qts = [], [], []
    for c in range(N_CORES):
        sl = slice(c * NQ_REAL, (c + 1) * NQ_REAL)
        qp = np.zeros((NQ_CORE, C), np.float32)
        qp[:NQ_REAL] = q[sl]
        rp = np.full((NQ_CORE, NOFF), 67.5, np.float32)
        rp[:NQ_REAL] = refpx[sl]
        qs.append(qp)
        cs.append(pack_consts(woff, rp, s))
        qts.append(np.ascontiguousarray(qp.T))
    return qs, cs, qts


_NC_CACHE = {}


def kernel(query, key, value, reference_points, spatial_shapes, W_off, b_off):
    from concourse.bass_utils import run_bass_kernel_spmd

    q, vp, refpx, woff, s, order = host_prep(query, value, reference_points,
                                             W_off, b_off)
    qs, cs, qts = shard(q, refpx, woff, s)

    if "nc" not in _NC_CACHE:
        nc = build_nc()
        nc.finalize()
        _NC_CACHE["nc"] = nc
    nc = _NC_CACHE["nc"]

    in_maps = [
        {"q": qs[c], "qt": qts[c], "consts": cs[c], "vp": vp}
        for c in range(N_CORES)
    ]
    res = run_bass_kernel_spmd(nc, in_maps, list(range(N_CORES)))
    srt = np.concatenate([res.results[c]["out"][:NQ_REAL] for c in range(N_CORES)], 0)
    out = np.empty_like(srt)
    out[order] = srt
    return out[None].astype(np.float32)


# revision 25
# speedup vs baseline: 1.1117x; 1.1006x over previous
"""Trainium2 Bass kernel for DeformableAttentionV2 — v2 (int16 table).

Sharding: queries split across 8 cores (2500 each, padded to 2560 = 20 tiles x
128); int16-quantized value feature maps + linear weights replicated per core.

Per core, per tile of 128 queries (query index on SBUF partitions):
  1. PE matmul: sampling offsets off[q, 36] = Q @ W_off  (Q^T staged by host).
  2. DVE+ACT: pixel coords = off + refpx (host folds 128*ref + b_off + 3.5),
     exact floor via the 2^23 magic trick, clamp to the zero-padded border,
     fractional bilinear weights, int32 row-pair gather indices.
  3. Indirect DMA gathers (one per point): 128 row-pairs x 1024 contiguous
     int16 (2KB descriptors) from the zero-padded pair-interleaved table
     vi16[2*136*136, 1024] holding v/s quantized to int16.
  4. kv[q, 18, 256] int16 (table units): 72 tensor_scalar corner products
     (int16 in/out -> DVE 4x packed mode, round-to-nearest verified on HW)
     + two big int16 tensor_tensor tree adds (2x) that are EXACT: bilinear
     weight pairs sum to <= 1 so integer sums stay within int16.
  5. logits via stt accum: lg[q, p] = <kv_p, q_fp32> in fp32; softmax with
     exp on ACT (scale=16*s via const AP, program stays input-independent).
  6. final combine: 18 fp16 ts products (4x) + fp16 pairwise tree adds (2x)
     on DVE, then * 1/sum(ex), fp32 store.

Zero padding implements grid_sample's padding_mode='zeros' exactly.
"""

import numpy as np
from contextlib import ExitStack

import concourse.bass as bass
import concourse.bacc as bacc_mod
import concourse.mybir as mybir
from concourse.bass import IndirectOffsetOnAxis
from concourse.tile import TileContext

# ---- hardcoded problem geometry ----
NQ = 20000
C = 256
NLEV = 2
NPT = 9
NLP = NLEV * NPT               # 18
H = W = 128
PAD = 4
HP = WP = H + 2 * PAD          # 136
LVL_ROWS = HP * WP             # 18496
VP_ROWS = NLEV * LVL_ROWS      # 36992
N_CORES = 8
NQ_CORE = 2560
NQ_REAL = 2500
P = 128
NOFF = NLP * 2                 # 36
MAGIC = float(np.float32(2.0 ** 23))
NT_FULL = NQ_CORE // P         # 20
NSC = 3                        # scalar consts: -16s, +16s, s
CONST_FREE = 2 * NOFF + NT_FULL * NOFF + NSC

F32 = mybir.dt.float32
F16 = mybir.dt.float16
I16 = mybir.dt.int16
I32 = mybir.dt.int32
AL = mybir.AluOpType
AX = mybir.AxisListType
AF = mybir.ActivationFunctionType

# (measured on HW: stt is always 1x; AP-scalar tensor_scalar caps at 2x;
# gpsimd tensor ops are too slow to offload to. kv/final are ts products +
# tree adds, with most products routed to the otherwise-idle ACT engine.)
ACT_PTS = 13         # kv products for points [0, ACT_PTS) run on ACT


def build_nc(n_tiles=NT_FULL, stop_after=None):
    nc = bacc_mod.Bacc()

    q_in = nc.declare_dram_parameter("q", [NQ_CORE, C], F32, isOutput=False)
    qt_in = nc.declare_dram_parameter("qt", [2 * P, NQ_CORE], F32, isOutput=False)
    consts_in = nc.declare_dram_parameter("consts", [P, CONST_FREE], F32,
                                          isOutput=False)
    vp_in = nc.declare_dram_parameter("vp", [VP_ROWS, 2 * C], I16, isOutput=False)
    out_ext = nc.declare_dram_parameter("out", [NQ_CORE, C], F32, isOutput=True)

    ctx = ExitStack()
    with ctx:
        tc = ctx.enter_context(TileContext(nc))

        const_pool = ctx.enter_context(tc.tile_pool(name="const", bufs=1))
        io_pool = ctx.enter_context(tc.tile_pool(name="io", bufs=2))
        g_pool = ctx.enter_context(tc.tile_pool(name="g", bufs=2))
        aprod_pool = ctx.enter_context(tc.tile_pool(name="aprod", bufs=2))
        dprod_pool = ctx.enter_context(tc.tile_pool(name="dprod", bufs=1))
        kv_pool = ctx.enter_context(tc.tile_pool(name="kv", bufs=1))
        sm_pool = ctx.enter_context(tc.tile_pool(name="sm", bufs=2))
        fin_pool = ctx.enter_context(tc.tile_pool(name="fin", bufs=1))
        ps_pool = ctx.enter_context(tc.tile_pool(name="ps", bufs=2, space="PSUM"))
        one_pool = ctx.enter_context(tc.tile_pool(name="one", bufs=1))

        consts_sb = const_pool.tile([P, CONST_FREE], F32, name="consts_sb")
        nc.sync.dma_start(out=consts_sb[:], in_=consts_in[:, :])
        w_sb = consts_sb[:, 0:2 * NOFF].rearrange("p (k j) -> p k j", k=2)
        refpx_full = consts_sb[:, 2 * NOFF:2 * NOFF + NT_FULL * NOFF].rearrange(
            "p (t j) -> p t j", j=NOFF)
        sc = 2 * NOFF + NT_FULL * NOFF
        neg16s = consts_sb[:, sc:sc + 1]
        pos16s = consts_sb[:, sc + 1:sc + 2]
        s_ap = consts_sb[:, sc + 2:sc + 3]

        scr_v = one_pool.tile([P, C], F32, name="scr_v")

        def emit_final(st):
            """Final combine for a finished tile: 18 fp16 ts products (4x),
            fp16 tree-add (2x), scale by 1/Z, fp32 store. All DVE."""
            kv, exs, rinv, t = st
            fpr = fin_pool.tile([P, NLP, C], F16, name="fpr")
            for pt in range(NLP):
                nc.vector.tensor_scalar(fpr[:, pt, :], kv[:, pt, :],
                                        exs[:, pt:pt + 1], None, AL.mult)
            # pairwise tree (fp16 2x): 18 -> 9 -> 4 (+1 leftover) -> 2 -> 1
            t9 = fin_pool.tile([P, 9, C], F16, name="t9")
            nc.vector.tensor_tensor(
                t9[:], fpr[:, 0:18:2, :], fpr[:, 1:18:2, :], AL.add)
            t4 = fin_pool.tile([P, 4, C], F16, name="t4")
            nc.vector.tensor_tensor(
                t4[:], t9[:, 0:8:2, :], t9[:, 1:8:2, :], AL.add)
            t2 = fin_pool.tile([P, 2, C], F16, name="t2")
            nc.vector.tensor_tensor(
                t2[:], t4[:, 0:4:2, :], t4[:, 1:4:2, :], AL.add)
            t1 = fin_pool.tile([P, C], F16, name="t1")
            nc.vector.tensor_tensor(t1[:], t2[:, 0, :], t2[:, 1, :], AL.add)
            t1b = fin_pool.tile([P, C], F16, name="t1b")
            nc.vector.tensor_tensor(t1b[:], t1[:], t9[:, 8, :], AL.add)
            out_f = fin_pool.tile([P, C], F32, name="out_f")
            nc.vector.tensor_scalar(out_f[:], t1b[:], rinv[:, 0:1],
                                    None, AL.mult)
            nc.sync.dma_start(out=out_ext[bass.ts(t, P), :], in_=out_f[:])

        def emit_head(t):
            """Stage tile t: input DMAs, offsets matmul, coords, gathers."""
            q_sb = io_pool.tile([P, C], F32, name="q_sb")
            nc.sync.dma_start(out=q_sb[:], in_=q_in[bass.ts(t, P), :])
            qt_sb = io_pool.tile([P, 2, P], F32, name="qt_sb")
            nc.sync.dma_start(
                out=qt_sb[:],
                in_=qt_in[:, :].rearrange("(k c) q -> c k q", k=2)[:, :, bass.ts(t, P)],
            )

            off_ps = ps_pool.tile([P, NOFF], F32, space="PSUM", name="off_ps")
            for k in range(2):
                nc.tensor.matmul(
                    out=off_ps[:], lhsT=qt_sb[:, k, :], rhs=w_sb[:, k, :],
                    start=(k == 0), stop=(k == 1))

            coord = sm_pool.tile([P, NOFF], F32, name="coord")
            nc.vector.tensor_tensor(coord[:], off_ps[:], refpx_full[:, t, :], AL.add)

            # floor via 2^23 magic
            rnd = sm_pool.tile([P, NOFF], F32, name="rnd")
            nc.vector.tensor_scalar(rnd[:], coord[:], MAGIC, MAGIC,
                                    AL.add, AL.subtract)
            gt = sm_pool.tile([P, NOFF], F32, name="gt")
            nc.vector.tensor_tensor(gt[:], rnd[:], coord[:], AL.is_gt)
            fl = sm_pool.tile([P, NOFF], F32, name="fl")
            nc.vector.tensor_tensor(fl[:], rnd[:], gt[:], AL.subtract)
            nc.vector.tensor_scalar(fl[:], fl[:], 0.0, 134.0, AL.max, AL.min)
            frac = sm_pool.tile([P, NOFF], F32, name="frac")
            nc.vector.tensor_tensor(frac[:], coord[:], fl[:], AL.subtract)

            wx01 = sm_pool.tile([P, NLP, 2], F32, name="wx01")
            wy01 = sm_pool.tile([P, NLP, 2], F32, name="wy01")
            for arr, xy in ((wx01, 0), (wy01, 1)):
                fr = frac[:, xy:NOFF:2]
                nc.vector.tensor_scalar(arr[:, :, 0], fr, -1.0, 1.0,
                                        AL.mult, AL.add)
                nc.vector.tensor_copy(arr[:, :, 1], fr)
            wcorn = sm_pool.tile([P, NLP, 4], F32, name="wcorn")
            nc.vector.tensor_tensor(
                wcorn[:].rearrange("p k (y x) -> p k y x", y=2, x=2),
                wy01[:, :, :, None].broadcast_to([P, NLP, 2, 2]),
                wx01[:, :, None, :].broadcast_to([P, NLP, 2, 2]),
                AL.mult,
            )

            basef = sm_pool.tile([P, NLP], F32, name="basef")
            nc.vector.tensor_scalar(basef[:], fl[:, 1:NOFF:2], 136.0, None, AL.mult)
            nc.vector.tensor_tensor(basef[:], basef[:], fl[:, 0:NOFF:2], AL.add)
            nc.vector.tensor_scalar(basef[:, NPT:], basef[:, NPT:],
                                    float(LVL_ROWS), None, AL.add)
            idxi = sm_pool.tile([P, NLP], I32, name="idxi")
            nc.vector.tensor_copy(idxi[:], basef[:])

            g_sb = g_pool.tile([P, NLP, 1024], I16, name="g_sb")
            for pt in range(NLP):
                nc.gpsimd.indirect_dma_start(
                    out=g_sb[:, pt, :], out_offset=None, in_=vp_in[:, :],
                    in_offset=IndirectOffsetOnAxis(ap=idxi[:, pt:pt + 1],
                                                   axis=0))
            return (g_sb, wcorn, q_sb, t)

        def g_slice(g_sb, pt, dy, dx):
            return g_sb[:, pt, 512 * dx + 256 * dy:512 * dx + 256 * dy + 256]

        def emit_act_products(head):
            """ACT's share of the kv corner products for a tile, emitted one
            tile ahead (after the previous tile's exp) so ACT never stalls."""
            g_sb, wcorn, _, _ = head
            aprod = aprod_pool.tile([P, ACT_PTS, 2, 2, C], I16, name="aprod")
            for pt in range(ACT_PTS):
                for dy in range(2):
                    for dx in range(2):
                        nc.scalar.activation(
                            aprod[:, pt, dy, dx, :], g_slice(g_sb, pt, dy, dx),
                            AF.Copy, scale=wcorn[:, pt, 2 * dy + dx:
                                                 2 * dy + dx + 1])
            return aprod

        def emit_compute(head, aprod):
            g_sb, wcorn, q_sb, t = head
            if stop_after == "gather":
                nc.vector.tensor_copy(scr_v[:], g_sb[:, 0, 0:C])
                nc.sync.dma_start(out=out_ext[bass.ts(t, P), :], in_=scr_v[:])
                return None

            # DVE's share of the corner products (points ACT_PTS..17)
            nd = NLP - ACT_PTS
            dprod = dprod_pool.tile([P, nd, 2, 2, C], I16, name="dprod")
            for i, pt in enumerate(range(ACT_PTS, NLP)):
                for dy in range(2):
                    for dx in range(2):
                        nc.vector.tensor_scalar(
                            dprod[:, i, dy, dx, :], g_slice(g_sb, pt, dy, dx),
                            wcorn[:, pt, 2 * dy + dx:2 * dy + dx + 1],
                            None, AL.mult)

            # kv accumulation: 3 chained int16 TT adds per buffer; every
            # partial sum is bounded by 32767 * (convex weight sum) so the
            # integer adds are exact.
            kv = kv_pool.tile([P, NLP, C], I16, name="kv")
            for buf, p0, np_ in ((aprod, 0, ACT_PTS), (dprod, ACT_PTS, nd)):
                kvs = kv[:, p0:p0 + np_, :]
                nc.vector.tensor_tensor(kvs, buf[:, :, 0, 0, :],
                                        buf[:, :, 0, 1, :], AL.add)
                nc.vector.tensor_tensor(kvs, kvs, buf[:, :, 1, 0, :], AL.add)
                nc.vector.tensor_tensor(kvs, kvs, buf[:, :, 1, 1, :], AL.add)

            if stop_after == "kv":
                nc.vector.tensor_copy(scr_v[:], kv[:, 0, :])
                nc.sync.dma_start(out=out_ext[bass.ts(t, P), :], in_=scr_v[:])
                return None

            # logits + softmax
            lg = sm_pool.tile([P, NLP], F32, name="lg")
            for pt in range(NLP):
                nc.vector.scalar_tensor_tensor(
                    out=scr_v[:], in0=kv[:, pt, :], scalar=1.0, in1=q_sb[:],
                    op0=AL.mult, op1=AL.mult, accum_out=lg[:, pt:pt + 1])
            mx = sm_pool.tile([P, 1], F32, name="mx")
            nc.vector.tensor_reduce(mx[:], lg[:], AX.X, AL.max)
            nbias = sm_pool.tile([P, 1], F32, name="nbias")
            nc.vector.tensor_scalar(nbias[:], mx[:], neg16s, None, AL.mult)
            ex = sm_pool.tile([P, NLP], F32, name="ex")
            nc.scalar.activation(ex[:], lg[:], AF.Exp, bias=nbias[:],
                                 scale=pos16s)
            rs = sm_pool.tile([P, 1], F32, name="rs")
            nc.vector.tensor_reduce(rs[:], ex[:], AX.X, AL.add)
            rinv = sm_pool.tile([P, 1], F32, name="rinv")
            nc.vector.reciprocal(rinv[:], rs[:])
            exs = sm_pool.tile([P, NLP], F32, name="exs")
            nc.scalar.activation(exs[:], ex[:], AF.Copy, scale=s_ap)
            return (kv, exs, rinv, t)

        head = emit_head(0)
        aprod = emit_act_products(head)
        for t in range(n_tiles):
            nxt = emit_head(t + 1) if t + 1 < n_tiles else None
            st = emit_compute(head, aprod)
            # ACT moves on to the NEXT tile's products right after exp/exs
            if nxt is not None:
                nxt_aprod = emit_act_products(nxt)
            if st is not None:
                emit_final(st)
            if nxt is not None:
                head, aprod = nxt, nxt_aprod
    return nc


# ---------------- host side ----------------

def host_prep(query, value, reference_points, W_off, b_off):
    rp_all = np.asarray(reference_points[0], dtype=np.float32)
    # sort queries by level-0 sample address so each gather's 128
    # descriptors hit a narrow, mostly-ascending band of the table
    # (HBM row locality); output is scattered back to original order.
    key = (np.rint(128.0 * rp_all[:, 0, 1]) * 136.0 + 128.0 * rp_all[:, 0, 0])
    order = np.argsort(key, kind="stable").astype(np.int64)
    q = np.ascontiguousarray(query[0][order], dtype=np.float32)
    rp = np.ascontiguousarray(rp_all[order])

    v = np.asarray(value, np.float32)
    s = float(np.abs(v).max() / 32766.0)
    vpad = np.zeros((NLEV, HP, WP, C), np.float32)
    vpad[:, PAD:PAD + H, PAD:PAD + W, :] = v.reshape(NLEV, H, W, C)
    vq = np.clip(np.rint(vpad * np.float32(1.0 / s)), -32767, 32767)
    vq = vq.astype(np.int16)
    vp = np.zeros((NLEV, HP, WP, 2, C), np.int16)
    vp[:, :, :, 0, :] = vq
    vp[:, :-1, :, 1, :] = vq[:, 1:]
    vp = np.ascontiguousarray(vp.reshape(VP_ROWS, 2 * C))

    b = np.asarray(b_off, np.float32).reshape(NLEV, NPT, 2)
    refpx = (np.float32(128.0) * rp[:, :, None, :] + b[None] + np.float32(3.5))
    refpx = np.ascontiguousarray(refpx.reshape(NQ, NOFF), np.float32)

    woff = np.ascontiguousarray(W_off, np.float32)
    return q, vp, refpx, woff, s, order


def pack_consts(woff, refpx_core, s):
    consts = np.empty((P, CONST_FREE), np.float32)
    consts[:, :2 * NOFF] = woff.reshape(2, P, NOFF).transpose(1, 0, 2).reshape(P, -1)
    consts[:, 2 * NOFF:2 * NOFF + NT_FULL * NOFF] = (
        refpx_core.reshape(NT_FULL, P, NOFF).transpose(1, 0, 2).reshape(P, -1))
    sc = 2 * NOFF + NT_FULL * NOFF
    consts[:, sc] = -16.0 * s
    consts[:, sc + 1] = 16.0 * s
    consts[:, sc + 2] = s
    return consts


def shard(q, refpx, woff, s):
    qs, cs, qts = [], [], []
    for c in range(N_CORES):
        sl = slice(c * NQ_REAL, (c + 1) * NQ_REAL)
        qp = np.zeros((NQ_CORE, C), np.float32)
        qp[:NQ_REAL] = q[sl]
        rp = np.full((NQ_CORE, NOFF), 67.5, np.float32)
        rp[:NQ_REAL] = refpx[sl]
        qs.append(qp)
        cs.append(pack_consts(woff, rp, s))
        qts.append(np.ascontiguousarray(qp.T))
    return qs, cs, qts


_NC_CACHE = {}


def kernel(query, key, value, reference_points, spatial_shapes, W_off, b_off):
    from concourse.bass_utils import run_bass_kernel_spmd

    q, vp, refpx, woff, s, order = host_prep(query, value, reference_points,
                                             W_off, b_off)
    qs, cs, qts = shard(q, refpx, woff, s)

    if "nc" not in _NC_CACHE:
        nc = build_nc()
        nc.finalize()
        _NC_CACHE["nc"] = nc
    nc = _NC_CACHE["nc"]

    in_maps = [
        {"q": qs[c], "qt": qts[c], "consts": cs[c], "vp": vp}
        for c in range(N_CORES)
    ]
    res = run_bass_kernel_spmd(nc, in_maps, list(range(N_CORES)))
    srt = np.concatenate([res.results[c]["out"][:NQ_REAL] for c in range(N_CORES)], 0)
    out = np.empty_like(srt)
    out[order] = srt
    return out[None].astype(np.float32)

